# revision 18
# baseline (speedup 1.0000x reference)
"""AutoCorrelation layer kernel for 8 Trainium2 NeuronCores.

Math note: the reference's rfft/irfft pair over the zero-padded head dim
computes a circular cross-correlation; its mean over all lags collapses
analytically to (sum_d q_proj) * (sum_d k_proj) per head.  So
corr_mean[b,l] = (1/(H*L)) * sum_h (q[b,l] @ WqS + bqS)_h * (k[b,l] @ WkS + bkS)_h
with WqS = Wq.reshape(D,H,DK).sum(-1).  Everything downstream (top-6,
softmax, gather, output projection) follows the reference directly.

Distribution (v2): batch-parallel preprocessing — core i computes the
corr/top-6/gather/agg for batch i only (f32 throughout: the 6th/7th
top-k gap can be ~1e-5 so bf16 corr flips selections), then a tiny
AllGather of the per-batch agg vectors [1,256] feeds the column-sharded
output projection.  Wp is pre-cast to bf16 on the host and streamed as
16 resident SBUF tiles whose DMAs all start at t=0; the big matmuls
consume them as soon as agg arrives.  v is fetched via a 6-row indexed
gather instead of a full 1MB load.
"""
import sys

sys.path.insert(0, "/opt/trn_rl_repo")

import numpy as np
import ml_dtypes
import concourse.bass as bass
import concourse.mybir as mybir
import concourse.tile as tile
from concourse import bacc
from concourse.bass_utils import run_bass_kernel_spmd
from concourse.masks import make_identity

F32 = mybir.dt.float32
BF16 = mybir.dt.bfloat16
U32 = mybir.dt.uint32

N_CORES = 8
B, L, D, H, DK = 8, 1024, 256, 8, 32
K_TOP = 6
NSH = (L * D) // N_CORES          # 32768 output cols per core
TILE_N = 2048
N_TILES = NSH // TILE_N           # 16
SUBS = TILE_N // 512              # 4
SCALE = 1.0 / (H * L)

TRACE = False          # test harness sets this for profiled runs
LAST_RESULT = None     # stashed BassKernelResults from the last kernel() call
MODE = "dp"            # "dp": batch-parallel preproc + AllGather; "repl": replicated preproc

_CACHE = {}


def _build_nc():
    nc = bacc.Bacc("TRN2", target_bir_lowering=False, debug=False, num_devices=N_CORES)

    qt_d = nc.dram_tensor("qt", [D, L], F32, kind="ExternalInput").ap()
    kt_d = nc.dram_tensor("kt", [D, L], F32, kind="ExternalInput").ap()
    v_d = nc.dram_tensor("v", [L, D], F32, kind="ExternalInput").ap()
    wq_d = nc.dram_tensor("wq", [D, D], F32, kind="ExternalInput").ap()
    wk_d = nc.dram_tensor("wk", [D, D], F32, kind="ExternalInput").ap()
    wv_d = nc.dram_tensor("wv", [D, D], F32, kind="ExternalInput").ap()
    bq_d = nc.dram_tensor("bq", [1, D], F32, kind="ExternalInput").ap()
    bk_d = nc.dram_tensor("bk", [1, D], F32, kind="ExternalInput").ap()
    bv_d = nc.dram_tensor("bv", [1, D], F32, kind="ExternalInput").ap()
    wp_d = nc.dram_tensor("wp", [D, NSH], BF16, kind="ExternalInput").ap()
    bp_d = nc.dram_tensor("bp", [1, 4 * 128 * 64 * 8], BF16, kind="ExternalInput").ap()
    out_d = nc.dram_tensor("out", [4 * 128, 64 * 8], BF16, kind="ExternalOutput").ap()

    with tile.TileContext(nc) as tc:
        with (
            tc.tile_pool(name="cst", bufs=1) as cst,
            tc.tile_pool(name="work", bufs=1) as work,
            tc.tile_pool(name="wpp", bufs=N_TILES) as wpp,
            tc.tile_pool(name="outp", bufs=2) as outp,
            tc.tile_pool(name="bpp", bufs=2) as bpp,
            tc.tile_pool(name="dr", bufs=1, space="DRAM") as dr,
            tc.tile_pool(name="ps_mm", bufs=1, space="PSUM") as ps_mm,
            tc.tile_pool(name="ps_tp", bufs=1, space="PSUM") as ps_tp,
            tc.tile_pool(name="ps_o", bufs=6, space="PSUM") as ps_o,
        ):
            # ---------------- phase 0: kick off all input DMAs ----------------
            # sync ring order: small weights -> qt/kt -> 16 wp tiles (16MB).
            wq_sb = cst.tile([128, 2, 256], F32)
            nc.sync.dma_start(wq_sb[:, :, :], wq_d.rearrange("(c p) d -> p c d", p=128))
            wk_sb = cst.tile([128, 2, 256], F32)
            nc.sync.dma_start(wk_sb[:, :, :], wk_d.rearrange("(c p) d -> p c d", p=128))
            wv_sb = cst.tile([128, 2, 256], F32)
            nc.sync.dma_start(wv_sb[:, :, :], wv_d.rearrange("(c p) d -> p c d", p=128))
            bq_sb = cst.tile([1, 256], F32)
            nc.sync.dma_start(bq_sb[:, :], bq_d)
            bk_sb = cst.tile([1, 256], F32)
            nc.sync.dma_start(bk_sb[:, :], bk_d)
            bv_sb = cst.tile([1, 256], F32)
            nc.sync.dma_start(bv_sb[:, :], bv_d)
            trq = work.tile([128, 2, L], F32)
            nc.sync.dma_start(trq[:, :, :], qt_d.rearrange("(c p) l -> p c l", p=128))
            trk = work.tile([128, 2, L], F32)
            nc.sync.dma_start(trk[:, :, :], kt_d.rearrange("(c p) l -> p c l", p=128))
            wpt = []
            for nt in range(N_TILES):
                ncol = slice(TILE_N * nt, TILE_N * (nt + 1))
                wp_t = wpp.tile([128, 2, TILE_N], BF16, tag="wp")
                eng = nc.sync if nt % 2 == 0 else nc.scalar
                eng.dma_start(
                    wp_t[:, :, :],
                    wp_d[:, ncol].rearrange("(c p) n -> p c n", p=128))
                wpt.append(wp_t)

            # ---------------- small constants ----------------
            ident8 = cst.tile([8, 8], F32)
            make_identity(nc, ident8[:, :])
            one1 = cst.tile([1, 1], F32)
            nc.vector.memset(one1[:, :], 1.0)
            sones = cst.tile([8, 1], F32)
            nc.vector.memset(sones[:, :], SCALE)

            # PE warm-up: the HAM clock gate needs ~3.4us of sustained PE
            # activity to lift the 1.2GHz cold throttle; burn it on junk
            # matmuls while the qt/kt DMAs are still in flight.
            ps_warm = ps_mm.tile([128, 512], F32, tag="mm")
            for _ in range(4):
                nc.tensor.matmul(ps_warm[:, 0:256], wq_sb[:, 0, 0:128], wq_sb[:, 0, :],
                                 start=True, stop=True)

            # head-sums of projection weights: WqS[d, h] = sum_z Wq[d, h*32+z]
            wqs = cst.tile([128, 2, 8], F32)
            nc.vector.reduce_sum(out=wqs[:, :, :],
                                 in_=wq_sb[:, :, :].rearrange("p c (h z) -> p c h z", z=DK),
                                 axis=mybir.AxisListType.X)
            wks = cst.tile([128, 2, 8], F32)
            nc.vector.reduce_sum(out=wks[:, :, :],
                                 in_=wk_sb[:, :, :].rearrange("p c (h z) -> p c h z", z=DK),
                                 axis=mybir.AxisListType.X)
            bqs_row = cst.tile([1, 8], F32)
            nc.vector.reduce_sum(out=bqs_row[:, :],
                                 in_=bq_sb[:, :].rearrange("o (h z) -> o h z", z=DK),
                                 axis=mybir.AxisListType.X)
            bks_row = cst.tile([1, 8], F32)
            nc.vector.reduce_sum(out=bks_row[:, :],
                                 in_=bk_sb[:, :].rearrange("o (h z) -> o h z", z=DK),
                                 axis=mybir.AxisListType.X)
            # [1,8] -> [8,1] via K=1 matmul against [1,1] ones
            bqs_ps = ps_tp.tile([8, 1], F32, tag="tp")
            nc.tensor.matmul(bqs_ps[:, :], bqs_row[:, :], one1[:, :], start=True, stop=True)
            bqs_vert = cst.tile([8, 1], F32)
            nc.vector.tensor_copy(bqs_vert[:, :], bqs_ps[:, :])
            bks_ps = ps_tp.tile([8, 1], F32, tag="tp")
            nc.tensor.matmul(bks_ps[:, :], bks_row[:, :], one1[:, :], start=True, stop=True)
            bks_vert = cst.tile([8, 1], F32)
            nc.vector.tensor_copy(bks_vert[:, :], bks_ps[:, :])

            # ---------------- per-batch corr (this core's batch only) ----------------
            xsT = {}
            for (tr, w_sum, bias_v, nm) in (
                (trq, wqs, bqs_vert, "q"),
                (trk, wks, bks_vert, "k"),
            ):
                xs = work.tile([8, L], F32, tag=f"{nm}sT")
                for half in range(2):
                    sl = slice(512 * half, 512 * (half + 1))
                    ps_x = ps_mm.tile([8, 512], F32, tag="mm")
                    nc.tensor.matmul(ps_x[:, :], w_sum[:, 0, :], tr[:, 0, sl], start=True, stop=False)
                    nc.tensor.matmul(ps_x[:, :], w_sum[:, 1, :], tr[:, 1, sl], start=False, stop=True)
                    nc.vector.tensor_scalar(
                        out=xs[:, sl], in0=ps_x[:, :],
                        scalar1=bias_v[:, 0:1], scalar2=None, op0=mybir.AluOpType.add)
                xsT[nm] = xs

            prod = work.tile([8, L], F32, tag="prod")
            nc.vector.tensor_mul(prod[:, :], xsT["q"][:, :], xsT["k"][:, :])
            r_sb = work.tile([1, L], F32, tag="qsT")
            for half in range(2):
                sl = slice(512 * half, 512 * (half + 1))
                ps_r = ps_mm.tile([1, 512], F32, tag="mm")
                nc.tensor.matmul(ps_r[:, :], sones[:, :], prod[:, sl], start=True, stop=True)
                nc.vector.tensor_copy(r_sb[:, sl], ps_r[:, :])

            # ---------------- top-6, softmax, 6-row gather of v ----------------
            topv = work.tile([1, 8], F32)
            nc.vector.max(topv[:, :], r_sb[:, :])
            topi = work.tile([1, 8], U32)
            nc.vector.max_index(topi[:, :], topv[:, :], r_sb[:, :])
            negm0 = work.tile([1, 1], F32)
            nc.vector.tensor_scalar_mul(negm0[:, :], topv[:, 0:1], -1.0)
            e_sb = work.tile([1, K_TOP], F32)
            nc.scalar.activation(e_sb[:, :], topv[:, 0:K_TOP],
                                 mybir.ActivationFunctionType.Exp,
                                 bias=negm0[:, 0:1], scale=1.0)
            z_sb = work.tile([1, 1], F32)
            nc.vector.reduce_sum(out=z_sb[:, :], in_=e_sb[:, :], axis=mybir.AxisListType.X)
            zinv = work.tile([1, 1], F32)
            nc.vector.reciprocal(zinv[:, :], z_sb[:, :])
            w_sb = work.tile([1, K_TOP], F32)
            nc.vector.tensor_scalar_mul(w_sb[:, :], e_sb[:, :], zinv[:, 0:1])

            # indices/weights -> columns via K=1 matmuls
            topi_f = work.tile([1, 8], F32)
            nc.vector.tensor_copy(topi_f[:, :], topi[:, :])
            idx_ps = ps_tp.tile([8, 1], F32, tag="tp")
            nc.tensor.matmul(idx_ps[:, :], topi_f[:, :], one1[:, :], start=True, stop=True)
            idx_colf = work.tile([8, 1], F32)
            nc.vector.tensor_copy(idx_colf[:, :], idx_ps[:, :])
            idx_col = work.tile([8, 1], U32)
            nc.vector.tensor_copy(idx_col[:, :], idx_colf[:, :])
            w_ps = ps_tp.tile([K_TOP, 1], F32, tag="tp")
            nc.tensor.matmul(w_ps[:, :], w_sb[:, :], one1[:, :], start=True, stop=True)
            w_col = work.tile([K_TOP, 1], F32)
            nc.vector.tensor_copy(w_col[:, :], w_ps[:, :])

            vrows = work.tile([K_TOP, 256], F32)
            nc.gpsimd.indirect_dma_start(
                out=vrows[:, :],
                out_offset=None,
                in_=v_d[:, :],
                in_offset=bass.IndirectOffsetOnAxis(ap=idx_col[0:K_TOP, 0:1], axis=0),
            )

            # vbar[1, 256] = w^T @ vrows
            vb_ps = ps_tp.tile([1, 256], F32, tag="tp")
            nc.tensor.matmul(vb_ps[:, :], w_col[:, :], vrows[:, :], start=True, stop=True)
            vbar = work.tile([1, 256], F32)
            nc.vector.tensor_copy(vbar[:, :], vb_ps[:, :])
            # vbar -> [128, 2] column chunks
            vbarT = work.tile([128, 2], F32)
            for m in range(2):
                pv = ps_tp.tile([128, 1], F32, tag="tp")
                nc.tensor.matmul(pv[:, :], vbar[0:1, 128 * m:128 * (m + 1)], one1[:, :],
                                 start=True, stop=True)
                nc.vector.tensor_copy(vbarT[:, m:m + 1], pv[:, :])

            # agg[d'] = sum_e Wv[e, d'] vbar[e] + bv[d']  -> [128, 2] (d' chunks)
            agg_sb = work.tile([128, 2], F32)
            for m in range(2):
                pa = ps_tp.tile([128, 1], F32, tag="tp")
                nc.tensor.matmul(pa[:, :], wv_sb[:, 0, 128 * m:128 * (m + 1)],
                                 vbarT[:, 0:1], start=True, stop=False)
                nc.tensor.matmul(pa[:, :], wv_sb[:, 1, 128 * m:128 * (m + 1)],
                                 vbarT[:, 1:2], start=False, stop=False)
                nc.tensor.matmul(pa[:, :], bv_sb[0:1, 128 * m:128 * (m + 1)],
                                 one1[:, :], start=False, stop=True)
                nc.vector.tensor_copy(agg_sb[:, m:m + 1], pa[:, :])

            # ---------------- AllGather agg -> [8, 256] ----------------
            agg_in = dr.tile([1, D], F32)
            nc.gpsimd.dma_start(
                agg_in[:, :].rearrange("o (m e) -> (o e) m", e=128), agg_sb[:, :])
            agg_out = dr.tile([B, D], F32)
            nc.gpsimd.collective_compute(
                "AllGather", mybir.AluOpType.bypass,
                replica_groups=[list(range(N_CORES))],
                ins=[agg_in[:, :].opt()], outs=[agg_out[:, :].opt()])
            aggf = cst.tile([8, 256], F32)
            nc.gpsimd.dma_start(aggf[:, :], agg_out[:, :])
            aggt_bf = cst.tile([128, 16], BF16)
            for m in range(2):
                pt = ps_tp.tile([128, 8], F32, tag="tp")
                nc.tensor.transpose(pt[:, :], aggf[0:8, 128 * m:128 * (m + 1)], ident8[:, :])
                nc.vector.tensor_copy(aggt_bf[:, 8 * m:8 * (m + 1)], pt[:, :])

            # ---------------- big output projection, transposed ----------------
            # outT[n, b] = sum_k Wp[k, n] agg[b, k]: Wp chunks are the
            # STATIONARY operand (M=128), aggt streams (N=8).  PSUM fills a
            # whole bank [128, 64, 8] before one full-width DVE drain that
            # also adds the (host-scrambled, pre-replicated) bias.  Host
            # unscrambles the [4, 128, 64, 8] output layout.
            bp_sb = cst.tile([128, 4, 64, 8], BF16)
            nc.sync.dma_start(bp_sb[:, :, :, :], bp_d.rearrange("o (p t c b) -> (o p) t c b", t=4, p=128, c=64))
            for t in range(4):
                ps = ps_o.tile([128, 64, 8], F32, tag="po")
                for c in range(64):
                    n0 = 8192 * t + 128 * c
                    wt = wpt[n0 // TILE_N]
                    co = n0 % TILE_N
                    nc.tensor.matmul(ps[:, c, :], wt[:, 0, co:co + 128], aggt_bf[:, 0:8],
                                     start=True, stop=False)
                    nc.tensor.matmul(ps[:, c, :], wt[:, 1, co:co + 128], aggt_bf[:, 8:16],
                                     start=False, stop=True)
                o_sbT = outp.tile([128, 64, 8], BF16)
                nc.vector.tensor_add(o_sbT[:, :, :], ps[:, :, :], bp_sb[:, t, :, :])
                nc.gpsimd.dma_start(out_d[128 * t:128 * (t + 1), :], o_sbT[:, :, :].rearrange("p c b -> p (c b)"))

    nc.finalize()
    return nc


def _build_nc_repl():
    """Replicated preprocessing: every core computes corr/top-6/agg for ALL
    8 batches (16MB redundant qt/kt read) so no cross-core collective is
    needed; the column-sharded projection starts as soon as local preproc
    finishes (~65us) instead of waiting ~90us for the AllGather."""
    nc = bacc.Bacc("TRN2", target_bir_lowering=False, debug=False, num_devices=N_CORES)

    qt_d = nc.dram_tensor("qt", [B * D, L], F32, kind="ExternalInput").ap()
    kt_d = nc.dram_tensor("kt", [B * D, L], F32, kind="ExternalInput").ap()
    v_d = nc.dram_tensor("v", [B * L, D], F32, kind="ExternalInput").ap()
    wq_d = nc.dram_tensor("wq", [D, D], F32, kind="ExternalInput").ap()
    wk_d = nc.dram_tensor("wk", [D, D], F32, kind="ExternalInput").ap()
    wv_d = nc.dram_tensor("wv", [D, D], F32, kind="ExternalInput").ap()
    bq_d = nc.dram_tensor("bq", [1, D], F32, kind="ExternalInput").ap()
    bk_d = nc.dram_tensor("bk", [1, D], F32, kind="ExternalInput").ap()
    bv_d = nc.dram_tensor("bv", [1, D], F32, kind="ExternalInput").ap()
    wp_d = nc.dram_tensor("wp", [D, NSH], BF16, kind="ExternalInput").ap()
    bp_d = nc.dram_tensor("bp", [1, 4 * 128 * 64 * 8], BF16, kind="ExternalInput").ap()
    out_d = nc.dram_tensor("out", [4 * 128, 64 * 8], BF16, kind="ExternalOutput").ap()

    with tile.TileContext(nc) as tc:
        with (
            tc.tile_pool(name="cst", bufs=1) as cst,
            tc.tile_pool(name="work", bufs=1) as work,
            tc.tile_pool(name="trp", bufs=2) as trp,
            tc.tile_pool(name="wpp", bufs=N_TILES) as wpp,
            tc.tile_pool(name="outp", bufs=2) as outp,
            tc.tile_pool(name="bpp", bufs=2) as bpp,
            tc.tile_pool(name="ps_mm", bufs=1, space="PSUM") as ps_mm,
            tc.tile_pool(name="ps_tp", bufs=1, space="PSUM") as ps_tp,
            tc.tile_pool(name="ps_o", bufs=6, space="PSUM") as ps_o,
        ):
            # ---- phase 0: all input DMAs; qt/kt tiles split across both
            # HWDGE rings ahead of the wp stream ----
            wq_sb = cst.tile([128, 2, 256], F32)
            nc.sync.dma_start(wq_sb[:, :, :], wq_d.rearrange("(c p) d -> p c d", p=128))
            wk_sb = cst.tile([128, 2, 256], F32)
            nc.sync.dma_start(wk_sb[:, :, :], wk_d.rearrange("(c p) d -> p c d", p=128))
            wv_sb = cst.tile([128, 2, 256], F32)
            nc.sync.dma_start(wv_sb[:, :, :], wv_d.rearrange("(c p) d -> p c d", p=128))
            bq_sb = cst.tile([1, 256], F32)
            nc.sync.dma_start(bq_sb[:, :], bq_d)
            bk_sb = cst.tile([1, 256], F32)
            nc.sync.dma_start(bk_sb[:, :], bk_d)
            bv_sb = cst.tile([1, 256], F32)
            nc.sync.dma_start(bv_sb[:, :], bv_d)
            # per-batch transposed q/k tiles: sync carries q, scalar carries k
            trqs, trks = [], []
            for b in range(B):
                trq = trp.tile([128, 2, L], F32, tag="trq")
                nc.sync.dma_start(
                    trq[:, :, :],
                    qt_d[D * b:D * (b + 1), :].rearrange("(c p) l -> p c l", p=128))
                trqs.append(trq)
                trk = trp.tile([128, 2, L], F32, tag="trk")
                nc.scalar.dma_start(
                    trk[:, :, :],
                    kt_d[D * b:D * (b + 1), :].rearrange("(c p) l -> p c l", p=128))
                trks.append(trk)
            wpt = []
            for nt in range(N_TILES):
                ncol = slice(TILE_N * nt, TILE_N * (nt + 1))
                wp_t = wpp.tile([128, 2, TILE_N], BF16, tag="wp")
                eng = nc.sync if nt % 2 == 0 else nc.scalar
                eng.dma_start(
                    wp_t[:, :, :],
                    wp_d[:, ncol].rearrange("(c p) n -> p c n", p=128))
                wpt.append(wp_t)

            # ---- small constants ----
            ident8 = cst.tile([8, 8], F32)
            make_identity(nc, ident8[:, :])
            one1 = cst.tile([1, 1], F32)
            nc.vector.memset(one1[:, :], 1.0)
            sones = cst.tile([8, 1], F32)
            nc.vector.memset(sones[:, :], SCALE)

            # PE warm-up while the first qt/kt tiles are in flight
            ps_warm = ps_mm.tile([128, 512], F32, tag="mm")
            for _ in range(4):
                nc.tensor.matmul(ps_warm[:, 0:256], wq_sb[:, 0, 0:128], wq_sb[:, 0, :],
                                 start=True, stop=True)

            wqs = cst.tile([128, 2, 8], F32)
            nc.vector.reduce_sum(out=wqs[:, :, :],
                                 in_=wq_sb[:, :, :].rearrange("p c (h z) -> p c h z", z=DK),
                                 axis=mybir.AxisListType.X)
            wks = cst.tile([128, 2, 8], F32)
            nc.vector.reduce_sum(out=wks[:, :, :],
                                 in_=wk_sb[:, :, :].rearrange("p c (h z) -> p c h z", z=DK),
                                 axis=mybir.AxisListType.X)
            bqs_row = cst.tile([1, 8], F32)
            nc.vector.reduce_sum(out=bqs_row[:, :],
                                 in_=bq_sb[:, :].rearrange("o (h z) -> o h z", z=DK),
                                 axis=mybir.AxisListType.X)
            bks_row = cst.tile([1, 8], F32)
            nc.vector.reduce_sum(out=bks_row[:, :],
                                 in_=bk_sb[:, :].rearrange("o (h z) -> o h z", z=DK),
                                 axis=mybir.AxisListType.X)
            bqs_ps = ps_tp.tile([8, 1], F32, tag="tp")
            nc.tensor.matmul(bqs_ps[:, :], bqs_row[:, :], one1[:, :], start=True, stop=True)
            bqs_vert = cst.tile([8, 1], F32)
            nc.vector.tensor_copy(bqs_vert[:, :], bqs_ps[:, :])
            bks_ps = ps_tp.tile([8, 1], F32, tag="tp")
            nc.tensor.matmul(bks_ps[:, :], bks_row[:, :], one1[:, :], start=True, stop=True)
            bks_vert = cst.tile([8, 1], F32)
            nc.vector.tensor_copy(bks_vert[:, :], bks_ps[:, :])

            # ---- per-batch corr, top-6, softmax, 6-row gather, vbar ----
            vbar8s = work.tile([1, 8, 256], F32)
            for b in range(B):
                xsT = {}
                for (tr, w_sum, bias_v, nm) in (
                    (trqs[b], wqs, bqs_vert, "q"),
                    (trks[b], wks, bks_vert, "k"),
                ):
                    xs = work.tile([8, L], F32, tag=f"{nm}sT")
                    for half in range(2):
                        sl = slice(512 * half, 512 * (half + 1))
                        ps_x = ps_mm.tile([8, 512], F32, tag="mm")
                        nc.tensor.matmul(ps_x[:, :], w_sum[:, 0, :], tr[:, 0, sl], start=True, stop=False)
                        nc.tensor.matmul(ps_x[:, :], w_sum[:, 1, :], tr[:, 1, sl], start=False, stop=True)
                        nc.vector.tensor_scalar(
                            out=xs[:, sl], in0=ps_x[:, :],
                            scalar1=bias_v[:, 0:1], scalar2=None, op0=mybir.AluOpType.add)
                    xsT[nm] = xs
                prod = work.tile([8, L], F32, tag="prod")
                nc.vector.tensor_mul(prod[:, :], xsT["q"][:, :], xsT["k"][:, :])
                r_sb = work.tile([1, L], F32, tag="qsT")
                for half in range(2):
                    sl = slice(512 * half, 512 * (half + 1))
                    ps_r = ps_mm.tile([1, 512], F32, tag="mm")
                    nc.tensor.matmul(ps_r[:, :], sones[:, :], prod[:, sl], start=True, stop=True)
                    nc.vector.tensor_copy(r_sb[:, sl], ps_r[:, :])

                topv = work.tile([1, 8], F32, tag="topv")
                nc.vector.max(topv[:, :], r_sb[:, :])
                topi = work.tile([1, 8], U32, tag="topi")
                nc.vector.max_index(topi[:, :], topv[:, :], r_sb[:, :])
                negm0 = work.tile([1, 1], F32, tag="negm0")
                nc.vector.tensor_scalar_mul(negm0[:, :], topv[:, 0:1], -1.0)
                e_sb = work.tile([1, K_TOP], F32, tag="e_sb")
                nc.scalar.activation(e_sb[:, :], topv[:, 0:K_TOP],
                                     mybir.ActivationFunctionType.Exp,
                                     bias=negm0[:, 0:1], scale=1.0)
                z_sb = work.tile([1, 1], F32, tag="z_sb")
                nc.vector.reduce_sum(out=z_sb[:, :], in_=e_sb[:, :], axis=mybir.AxisListType.X)
                zinv = work.tile([1, 1], F32, tag="zinv")
                nc.vector.reciprocal(zinv[:, :], z_sb[:, :])
                w_sb = work.tile([1, K_TOP], F32, tag="w_sb")
                nc.vector.tensor_scalar_mul(w_sb[:, :], e_sb[:, :], zinv[:, 0:1])

                # indices (+1024*b since v spans all batches) -> gather 6 rows
                topi_f = work.tile([1, 8], F32, tag="topi_f")
                nc.vector.tensor_copy(topi_f[:, :], topi[:, :])
                nc.vector.tensor_scalar_add(topi_f[:, :], topi_f[:, :], float(L * b))
                idx_ps = ps_tp.tile([8, 1], F32, tag="tp")
                nc.tensor.matmul(idx_ps[:, :], topi_f[:, :], one1[:, :], start=True, stop=True)
                idx_colf = work.tile([8, 1], F32, tag="idx_colf", bufs=2)
                nc.vector.tensor_copy(idx_colf[:, :], idx_ps[:, :])
                idx_col = work.tile([8, 1], U32, tag="idx_col", bufs=2)
                nc.vector.tensor_copy(idx_col[:, :], idx_colf[:, :])
                w_ps = ps_tp.tile([K_TOP, 1], F32, tag="tp")
                nc.tensor.matmul(w_ps[:, :], w_sb[:, :], one1[:, :], start=True, stop=True)
                w_col = work.tile([K_TOP, 1], F32, tag="w_col", bufs=2)
                nc.vector.tensor_copy(w_col[:, :], w_ps[:, :])
                vrows = work.tile([K_TOP, 256], F32, tag="vrows", bufs=2)
                nc.gpsimd.indirect_dma_start(
                    out=vrows[:, :],
                    out_offset=None,
                    in_=v_d[:, :],
                    in_offset=bass.IndirectOffsetOnAxis(ap=idx_col[0:K_TOP, 0:1], axis=0),
                )
                vb_ps = ps_tp.tile([1, 256], F32, tag="tp")
                nc.tensor.matmul(vb_ps[:, :], w_col[:, :], vrows[:, :], start=True, stop=True)
                nc.vector.tensor_copy(vbar8s[:, b, :], vb_ps[:, :])

            # ---- partition-spread vbar, then Wv projection ----
            vbar8 = work.tile([8, 256], F32)
            nc.gpsimd.dma_start(vbar8[:, :], vbar8s[:, :, :].rearrange("o b d -> (o b) d"))
            vbarT = work.tile([128, 2, 8], F32)
            for m in range(2):
                pt = ps_tp.tile([128, 8], F32, tag="tp")
                nc.tensor.transpose(pt[:, :], vbar8[0:8, 128 * m:128 * (m + 1)], ident8[:, :])
                nc.vector.tensor_copy(vbarT[:, m, :], pt[:, :])
            # aggf[8, 256] = vbar @ Wv + bv
            ps_a = ps_mm.tile([8, 512], F32, tag="mm")
            nc.tensor.matmul(ps_a[:, 0:256], vbarT[:, 0, :], wv_sb[:, 0, :], start=True, stop=False)
            nc.tensor.matmul(ps_a[:, 0:256], vbarT[:, 1, :], wv_sb[:, 1, :], start=False, stop=False)
            ones8m = cst.tile([1, 8], F32)
            nc.vector.memset(ones8m[:, :], 1.0)
            nc.tensor.matmul(ps_a[:, 0:256], ones8m[:, :], bv_sb[:, :], start=False, stop=True)
            aggf = cst.tile([8, 256], F32)
            nc.vector.tensor_copy(aggf[:, :], ps_a[:, 0:256])
            aggt_bf = cst.tile([128, 16], BF16)
            for m in range(2):
                pt2 = ps_tp.tile([128, 8], F32, tag="tp")
                nc.tensor.transpose(pt2[:, :], aggf[0:8, 128 * m:128 * (m + 1)], ident8[:, :])
                nc.vector.tensor_copy(aggt_bf[:, 8 * m:8 * (m + 1)], pt2[:, :])

            # ---- big output projection, transposed (see dp build) ----
            bp_sb = cst.tile([128, 4, 64, 8], BF16)
            nc.sync.dma_start(bp_sb[:, :, :, :], bp_d.rearrange("o (p t c b) -> (o p) t c b", t=4, p=128, c=64))
            for t in range(4):
                ps = ps_o.tile([128, 64, 8], F32, tag="po")
                for c in range(64):
                    n0 = 8192 * t + 128 * c
                    wt = wpt[n0 // TILE_N]
                    co = n0 % TILE_N
                    nc.tensor.matmul(ps[:, c, :], wt[:, 0, co:co + 128], aggt_bf[:, 0:8],
                                     start=True, stop=False)
                    nc.tensor.matmul(ps[:, c, :], wt[:, 1, co:co + 128], aggt_bf[:, 8:16],
                                     start=False, stop=True)
                o_sbT = outp.tile([128, 64, 8], BF16)
                nc.vector.tensor_add(o_sbT[:, :, :], ps[:, :, :], bp_sb[:, t, :, :])
                nc.gpsimd.dma_start(out_d[128 * t:128 * (t + 1), :], o_sbT[:, :, :].rearrange("p c b -> p (c b)"))

    nc.finalize()
    return nc


def _get_nc():
    if "nc" not in _CACHE:
        _CACHE["nc"] = _build_nc_repl() if MODE == "repl" else _build_nc()
    return _CACHE["nc"]


def kernel(queries, keys, values, Wq, bq, Wk, bk, Wv, bv, Wp, bp):
    queries = np.asarray(queries, np.float32)
    keys = np.asarray(keys, np.float32)
    values = np.asarray(values, np.float32)
    Wq = np.ascontiguousarray(np.asarray(Wq, np.float32))
    Wk = np.ascontiguousarray(np.asarray(Wk, np.float32))
    Wv = np.ascontiguousarray(np.asarray(Wv, np.float32))
    bq = np.asarray(bq, np.float32).reshape(1, D)
    bk = np.asarray(bk, np.float32).reshape(1, D)
    bv = np.asarray(bv, np.float32).reshape(1, D)
    Wp = np.asarray(Wp, np.float32)
    bp = np.asarray(bp, np.float32)

    nc = _get_nc()
    qT = np.ascontiguousarray(queries.transpose(0, 2, 1))   # [B, D, L]
    kT = np.ascontiguousarray(keys.transpose(0, 2, 1))
    Wp_bf = np.asarray(Wp, dtype=ml_dtypes.bfloat16)
    in_maps = []
    for i in range(N_CORES):
        cols = slice(NSH * i, NSH * (i + 1))
        # bias pre-scrambled to the transposed-output layout [p, t, c, b]
        # (value at flat (t, p, c, b) = bp[8192 t + 128 c + p], replicated over b)
        bp_shard = np.asarray(bp[cols], np.float32).reshape(4, 64, 128)
        bp_scr = np.broadcast_to(
            bp_shard.transpose(2, 0, 1)[:, :, :, None], (128, 4, 64, 8))
        m = {
            "wq": Wq, "wk": Wk, "wv": Wv,
            "bq": bq, "bk": bk, "bv": bv,
            "wp": np.ascontiguousarray(Wp_bf[:, cols]),
            "bp": np.ascontiguousarray(
                np.asarray(bp_scr, dtype=ml_dtypes.bfloat16)).reshape(1, 4 * 128 * 64 * 8),
        }
        if MODE == "repl":
            m.update({"qt": qT.reshape(B * D, L), "kt": kT.reshape(B * D, L),
                      "v": values.reshape(B * L, D)})
        else:
            m.update({"qt": qT[i], "kt": kT[i], "v": values[i]})
        in_maps.append(m)
    res = run_bass_kernel_spmd(nc, in_maps, core_ids=list(range(N_CORES)), trace=TRACE)
    global LAST_RESULT
    LAST_RESULT = res
    shards = []
    for i in range(N_CORES):
        buf = np.asarray(res.results[i]["out"], np.float32)
        if buf.shape == (4 * 128, 64 * 8):
            # transposed layout: buf[(t p), (c b)] -> shard[b, 8192 t + 128 c + p]
            shards.append(buf.reshape(4, 128, 64, 8).transpose(3, 0, 2, 1).reshape(B, NSH))
        else:
            shards.append(buf)
    out = np.concatenate(shards, axis=1)
    return out.reshape(B, L, D)


# revision 20
# speedup vs baseline: 1.1939x; 1.1939x over previous
"""AutoCorrelation layer kernel for 8 Trainium2 NeuronCores.

Math note: the reference's rfft/irfft pair over the zero-padded head dim
computes a circular cross-correlation; its mean over all lags collapses
analytically to (sum_d q_proj) * (sum_d k_proj) per head.  So
corr_mean[b,l] = (1/(H*L)) * sum_h (q[b,l] @ WqS + bqS)_h * (k[b,l] @ WkS + bkS)_h
with WqS = Wq.reshape(D,H,DK).sum(-1).  Everything downstream (top-6,
softmax, gather, output projection) follows the reference directly.

Distribution (v2): batch-parallel preprocessing — core i computes the
corr/top-6/gather/agg for batch i only (f32 throughout: the 6th/7th
top-k gap can be ~1e-5 so bf16 corr flips selections), then a tiny
AllGather of the per-batch agg vectors [1,256] feeds the column-sharded
output projection.  Wp is pre-cast to bf16 on the host and streamed as
16 resident SBUF tiles whose DMAs all start at t=0; the big matmuls
consume them as soon as agg arrives.  v is fetched via a 6-row indexed
gather instead of a full 1MB load.
"""
import sys

sys.path.insert(0, "/opt/trn_rl_repo")

import numpy as np
import ml_dtypes
import concourse.bass as bass
import concourse.mybir as mybir
import concourse.tile as tile
from concourse import bacc
from concourse.bass_utils import run_bass_kernel_spmd
from concourse.masks import make_identity

F32 = mybir.dt.float32
BF16 = mybir.dt.bfloat16
U32 = mybir.dt.uint32

N_CORES = 8
B, L, D, H, DK = 8, 1024, 256, 8, 32
K_TOP = 6
NSH = (L * D) // N_CORES          # 32768 output cols per core
TILE_N = 2048
N_TILES = NSH // TILE_N           # 16
SUBS = TILE_N // 512              # 4
SCALE = 1.0 / (H * L)

TRACE = False          # test harness sets this for profiled runs
LAST_RESULT = None     # stashed BassKernelResults from the last kernel() call
MODE = "dp"            # "dp": batch-parallel preproc + AllGather; "repl": replicated preproc

_CACHE = {}


def _build_nc():
    nc = bacc.Bacc("TRN2", target_bir_lowering=False, debug=False, num_devices=N_CORES)

    qt0_d = nc.dram_tensor("qt0", [D, L], F32, kind="ExternalInput").ap()
    kt0_d = nc.dram_tensor("kt0", [D, L], F32, kind="ExternalInput").ap()
    v0_d = nc.dram_tensor("v0", [L, D], F32, kind="ExternalInput").ap()
    qt1_d = nc.dram_tensor("qt1", [D, L], F32, kind="ExternalInput").ap()
    kt1_d = nc.dram_tensor("kt1", [D, L], F32, kind="ExternalInput").ap()
    v1_d = nc.dram_tensor("v1", [L, D], F32, kind="ExternalInput").ap()
    wq_d = nc.dram_tensor("wq", [D, D], F32, kind="ExternalInput").ap()
    wk_d = nc.dram_tensor("wk", [D, D], F32, kind="ExternalInput").ap()
    wv_d = nc.dram_tensor("wv", [D, D], F32, kind="ExternalInput").ap()
    bq_d = nc.dram_tensor("bq", [1, D], F32, kind="ExternalInput").ap()
    bk_d = nc.dram_tensor("bk", [1, D], F32, kind="ExternalInput").ap()
    bv_d = nc.dram_tensor("bv", [1, D], F32, kind="ExternalInput").ap()
    wp_d = nc.dram_tensor("wp", [D, NSH], BF16, kind="ExternalInput").ap()
    bp_d = nc.dram_tensor("bp", [1, 4 * 128 * 64 * 8], BF16, kind="ExternalInput").ap()
    out_d = nc.dram_tensor("out", [4 * 128, 64 * 8], BF16, kind="ExternalOutput").ap()

    with tile.TileContext(nc) as tc:
        with (
            tc.tile_pool(name="cst", bufs=1) as cst,
            tc.tile_pool(name="work", bufs=1) as work,
            tc.tile_pool(name="wpp", bufs=N_TILES) as wpp,
            tc.tile_pool(name="outp", bufs=2) as outp,
            tc.tile_pool(name="bpp", bufs=2) as bpp,
            tc.tile_pool(name="dr", bufs=1, space="DRAM") as dr,
            tc.tile_pool(name="ps_mm", bufs=1, space="PSUM") as ps_mm,
            tc.tile_pool(name="ps_tp", bufs=1, space="PSUM") as ps_tp,
            tc.tile_pool(name="ps_o", bufs=6, space="PSUM") as ps_o,
        ):
            # ---------------- phase 0: kick off all input DMAs ----------------
            # sync ring order: small weights -> qt/kt -> 16 wp tiles (16MB).
            wq_sb = cst.tile([128, 2, 256], F32)
            nc.sync.dma_start(wq_sb[:, :, :], wq_d.rearrange("(c p) d -> p c d", p=128))
            wk_sb = cst.tile([128, 2, 256], F32)
            nc.sync.dma_start(wk_sb[:, :, :], wk_d.rearrange("(c p) d -> p c d", p=128))
            wv_sb = cst.tile([128, 2, 256], F32)
            nc.sync.dma_start(wv_sb[:, :, :], wv_d.rearrange("(c p) d -> p c d", p=128))
            bq_sb = cst.tile([1, 256], F32)
            nc.sync.dma_start(bq_sb[:, :], bq_d)
            bk_sb = cst.tile([1, 256], F32)
            nc.sync.dma_start(bk_sb[:, :], bk_d)
            bv_sb = cst.tile([1, 256], F32)
            nc.sync.dma_start(bv_sb[:, :], bv_d)
            trq0 = work.tile([128, 2, L], F32)
            nc.sync.dma_start(trq0[:, :, :], qt0_d.rearrange("(c p) l -> p c l", p=128))
            trk0 = work.tile([128, 2, L], F32)
            nc.scalar.dma_start(trk0[:, :, :], kt0_d.rearrange("(c p) l -> p c l", p=128))
            trq1 = work.tile([128, 2, L], F32)
            nc.sync.dma_start(trq1[:, :, :], qt1_d.rearrange("(c p) l -> p c l", p=128))
            trk1 = work.tile([128, 2, L], F32)
            nc.scalar.dma_start(trk1[:, :, :], kt1_d.rearrange("(c p) l -> p c l", p=128))
            wpt = []
            for nt in range(N_TILES):
                ncol = slice(TILE_N * nt, TILE_N * (nt + 1))
                wp_t = wpp.tile([128, 2, TILE_N], BF16, tag="wp")
                eng = nc.sync if nt % 2 == 0 else nc.scalar
                eng.dma_start(
                    wp_t[:, :, :],
                    wp_d[:, ncol].rearrange("(c p) n -> p c n", p=128))
                wpt.append(wp_t)

            # ---------------- small constants ----------------
            ident8 = cst.tile([8, 8], F32)
            make_identity(nc, ident8[:, :])
            one1 = cst.tile([1, 1], F32)
            nc.vector.memset(one1[:, :], 1.0)
            sones = cst.tile([8, 1], F32)
            nc.vector.memset(sones[:, :], SCALE)

            # PE warm-up: the HAM clock gate needs ~3.4us of sustained PE
            # activity to lift the 1.2GHz cold throttle; burn it on junk
            # matmuls while the qt/kt DMAs are still in flight.
            ps_warm = ps_mm.tile([128, 512], F32, tag="mm")
            for _ in range(4):
                nc.tensor.matmul(ps_warm[:, 0:256], wq_sb[:, 0, 0:128], wq_sb[:, 0, :],
                                 start=True, stop=True)

            # head-sums of projection weights: WqS[d, h] = sum_z Wq[d, h*32+z]
            wqs = cst.tile([128, 2, 8], F32)
            nc.vector.reduce_sum(out=wqs[:, :, :],
                                 in_=wq_sb[:, :, :].rearrange("p c (h z) -> p c h z", z=DK),
                                 axis=mybir.AxisListType.X)
            wks = cst.tile([128, 2, 8], F32)
            nc.vector.reduce_sum(out=wks[:, :, :],
                                 in_=wk_sb[:, :, :].rearrange("p c (h z) -> p c h z", z=DK),
                                 axis=mybir.AxisListType.X)
            bqs_row = cst.tile([1, 8], F32)
            nc.vector.reduce_sum(out=bqs_row[:, :],
                                 in_=bq_sb[:, :].rearrange("o (h z) -> o h z", z=DK),
                                 axis=mybir.AxisListType.X)
            bks_row = cst.tile([1, 8], F32)
            nc.vector.reduce_sum(out=bks_row[:, :],
                                 in_=bk_sb[:, :].rearrange("o (h z) -> o h z", z=DK),
                                 axis=mybir.AxisListType.X)
            # [1,8] -> [8,1] via K=1 matmul against [1,1] ones
            bqs_ps = ps_tp.tile([8, 1], F32, tag="tp")
            nc.tensor.matmul(bqs_ps[:, :], bqs_row[:, :], one1[:, :], start=True, stop=True)
            bqs_vert = cst.tile([8, 1], F32)
            nc.vector.tensor_copy(bqs_vert[:, :], bqs_ps[:, :])
            bks_ps = ps_tp.tile([8, 1], F32, tag="tp")
            nc.tensor.matmul(bks_ps[:, :], bks_row[:, :], one1[:, :], start=True, stop=True)
            bks_vert = cst.tile([8, 1], F32)
            nc.vector.tensor_copy(bks_vert[:, :], bks_ps[:, :])

            # ------------- per-batch corr for the TWO local batches -------------
            agg_sb2 = work.tile([128, 2, 2], F32)
            for bl, (trq, trk, vcur_d) in enumerate(
                ((trq0, trk0, v0_d), (trq1, trk1, v1_d))):
              xsT = {}
              for (tr, w_sum, bias_v, nm) in (
                (trq, wqs, bqs_vert, "q"),
                (trk, wks, bks_vert, "k"),
              ):
                xs = work.tile([8, L], F32, tag=f"{nm}sT")
                for half in range(2):
                    sl = slice(512 * half, 512 * (half + 1))
                    ps_x = ps_mm.tile([8, 512], F32, tag="mm")
                    nc.tensor.matmul(ps_x[:, :], w_sum[:, 0, :], tr[:, 0, sl], start=True, stop=False)
                    nc.tensor.matmul(ps_x[:, :], w_sum[:, 1, :], tr[:, 1, sl], start=False, stop=True)
                    nc.vector.tensor_scalar(
                        out=xs[:, sl], in0=ps_x[:, :],
                        scalar1=bias_v[:, 0:1], scalar2=None, op0=mybir.AluOpType.add)
                xsT[nm] = xs

              prod = work.tile([8, L], F32, tag="prod")
              nc.vector.tensor_mul(prod[:, :], xsT["q"][:, :], xsT["k"][:, :])
              r_sb = work.tile([1, L], F32, tag="qsT")
              for half in range(2):
                sl = slice(512 * half, 512 * (half + 1))
                ps_r = ps_mm.tile([1, 512], F32, tag="mm")
                nc.tensor.matmul(ps_r[:, :], sones[:, :], prod[:, sl], start=True, stop=True)
                nc.vector.tensor_copy(r_sb[:, sl], ps_r[:, :])

              topv = work.tile([1, 8], F32, tag="topv")
              nc.vector.max(topv[:, :], r_sb[:, :])
              topi = work.tile([1, 8], U32, tag="topi")
              nc.vector.max_index(topi[:, :], topv[:, :], r_sb[:, :])
              negm0 = work.tile([1, 1], F32, tag="negm0")
              nc.vector.tensor_scalar_mul(negm0[:, :], topv[:, 0:1], -1.0)
              e_sb = work.tile([1, K_TOP], F32, tag="e_sb")
              nc.scalar.activation(e_sb[:, :], topv[:, 0:K_TOP],
                                   mybir.ActivationFunctionType.Exp,
                                   bias=negm0[:, 0:1], scale=1.0)
              z_sb = work.tile([1, 1], F32, tag="z_sb")
              nc.vector.reduce_sum(out=z_sb[:, :], in_=e_sb[:, :], axis=mybir.AxisListType.X)
              zinv = work.tile([1, 1], F32, tag="zinv")
              nc.vector.reciprocal(zinv[:, :], z_sb[:, :])
              w_sb = work.tile([1, K_TOP], F32, tag="w_sb")
              nc.vector.tensor_scalar_mul(w_sb[:, :], e_sb[:, :], zinv[:, 0:1])

              topi_f = work.tile([1, 8], F32, tag="topi_f")
              nc.vector.tensor_copy(topi_f[:, :], topi[:, :])
              idx_ps = ps_tp.tile([8, 1], F32, tag="tp")
              nc.tensor.matmul(idx_ps[:, :], topi_f[:, :], one1[:, :], start=True, stop=True)
              idx_colf = work.tile([8, 1], F32, tag="idx_colf")
              nc.vector.tensor_copy(idx_colf[:, :], idx_ps[:, :])
              idx_col = work.tile([8, 1], U32, tag="idx_col")
              nc.vector.tensor_copy(idx_col[:, :], idx_colf[:, :])
              w_ps = ps_tp.tile([K_TOP, 1], F32, tag="tp")
              nc.tensor.matmul(w_ps[:, :], w_sb[:, :], one1[:, :], start=True, stop=True)
              w_col = work.tile([K_TOP, 1], F32, tag="w_col")
              nc.vector.tensor_copy(w_col[:, :], w_ps[:, :])

              vrows = work.tile([K_TOP, 256], F32, tag="vrows")
              nc.gpsimd.indirect_dma_start(
                out=vrows[:, :],
                out_offset=None,
                in_=vcur_d[:, :],
                in_offset=bass.IndirectOffsetOnAxis(ap=idx_col[0:K_TOP, 0:1], axis=0),
              )

              vb_ps = ps_tp.tile([1, 256], F32, tag="tp")
              nc.tensor.matmul(vb_ps[:, :], w_col[:, :], vrows[:, :], start=True, stop=True)
              vbar = work.tile([1, 256], F32, tag="vbar")
              nc.vector.tensor_copy(vbar[:, :], vb_ps[:, :])
              vbarT = work.tile([128, 2], F32, tag="vbarT")
              for m in range(2):
                pv = ps_tp.tile([128, 1], F32, tag="tp")
                nc.tensor.matmul(pv[:, :], vbar[0:1, 128 * m:128 * (m + 1)], one1[:, :],
                                 start=True, stop=True)
                nc.vector.tensor_copy(vbarT[:, m:m + 1], pv[:, :])

              for m in range(2):
                pa = ps_tp.tile([128, 1], F32, tag="tp")
                nc.tensor.matmul(pa[:, :], wv_sb[:, 0, 128 * m:128 * (m + 1)],
                                 vbarT[:, 0:1], start=True, stop=False)
                nc.tensor.matmul(pa[:, :], wv_sb[:, 1, 128 * m:128 * (m + 1)],
                                 vbarT[:, 1:2], start=False, stop=False)
                nc.tensor.matmul(pa[:, :], bv_sb[0:1, 128 * m:128 * (m + 1)],
                                 one1[:, :], start=False, stop=True)
                nc.vector.tensor_copy(agg_sb2[:, m, bl:bl + 1], pa[:, :])

            # ------- 4-rank AllGather: [2, 256] local aggs -> [8, 256] -------
            # cores {2g, 2g+1} both hold batches {2g, 2g+1}; groups span one
            # core of each pair so every core's output rows land in batch order.
            agg_in = dr.tile([2, D], F32)
            for bl in range(2):
                nc.gpsimd.dma_start(
                    agg_in[bl:bl + 1, :].rearrange("o (m e) -> (o e) m", e=128),
                    agg_sb2[:, :, bl])
            agg_out = dr.tile([B, D], F32)
            nc.gpsimd.collective_compute(
                "AllGather", mybir.AluOpType.bypass,
                replica_groups=[[0, 2, 4, 6], [1, 3, 5, 7]],
                ins=[agg_in[:, :].opt()], outs=[agg_out[:, :].opt()])
            aggf = cst.tile([8, 256], F32)
            nc.gpsimd.dma_start(aggf[:, :], agg_out[:, :])
            aggt_bf = cst.tile([128, 16], BF16)
            for m in range(2):
                pt = ps_tp.tile([128, 8], F32, tag="tp")
                nc.tensor.transpose(pt[:, :], aggf[0:8, 128 * m:128 * (m + 1)], ident8[:, :])
                nc.vector.tensor_copy(aggt_bf[:, 8 * m:8 * (m + 1)], pt[:, :])

            # ---------------- big output projection, transposed ----------------
            # outT[n, b] = sum_k Wp[k, n] agg[b, k]: Wp chunks are the
            # STATIONARY operand (M=128), aggt streams (N=8).  PSUM fills a
            # whole bank [128, 64, 8] before one full-width DVE drain that
            # also adds the (host-scrambled, pre-replicated) bias.  Host
            # unscrambles the [4, 128, 64, 8] output layout.
            bp_sb = cst.tile([128, 4, 64, 8], BF16)
            nc.sync.dma_start(bp_sb[:, :, :, :], bp_d.rearrange("o (p t c b) -> (o p) t c b", t=4, p=128, c=64))
            for t in range(4):
                ps = ps_o.tile([128, 64, 8], F32, tag="po")
                for c in range(64):
                    n0 = 8192 * t + 128 * c
                    wt = wpt[n0 // TILE_N]
                    co = n0 % TILE_N
                    nc.tensor.matmul(ps[:, c, :], wt[:, 0, co:co + 128], aggt_bf[:, 0:8],
                                     start=True, stop=False)
                    nc.tensor.matmul(ps[:, c, :], wt[:, 1, co:co + 128], aggt_bf[:, 8:16],
                                     start=False, stop=True)
                o_sbT = outp.tile([128, 64, 8], BF16)
                nc.vector.tensor_add(o_sbT[:, :, :], ps[:, :, :], bp_sb[:, t, :, :])
                nc.gpsimd.dma_start(out_d[128 * t:128 * (t + 1), :], o_sbT[:, :, :].rearrange("p c b -> p (c b)"))

    nc.finalize()
    return nc


def _build_nc_repl():
    """Replicated preprocessing: every core computes corr/top-6/agg for ALL
    8 batches (16MB redundant qt/kt read) so no cross-core collective is
    needed; the column-sharded projection starts as soon as local preproc
    finishes (~65us) instead of waiting ~90us for the AllGather."""
    nc = bacc.Bacc("TRN2", target_bir_lowering=False, debug=False, num_devices=N_CORES)

    qt_d = nc.dram_tensor("qt", [B * D, L], F32, kind="ExternalInput").ap()
    kt_d = nc.dram_tensor("kt", [B * D, L], F32, kind="ExternalInput").ap()
    v_d = nc.dram_tensor("v", [B * L, D], F32, kind="ExternalInput").ap()
    wq_d = nc.dram_tensor("wq", [D, D], F32, kind="ExternalInput").ap()
    wk_d = nc.dram_tensor("wk", [D, D], F32, kind="ExternalInput").ap()
    wv_d = nc.dram_tensor("wv", [D, D], F32, kind="ExternalInput").ap()
    bq_d = nc.dram_tensor("bq", [1, D], F32, kind="ExternalInput").ap()
    bk_d = nc.dram_tensor("bk", [1, D], F32, kind="ExternalInput").ap()
    bv_d = nc.dram_tensor("bv", [1, D], F32, kind="ExternalInput").ap()
    wp_d = nc.dram_tensor("wp", [D, NSH], BF16, kind="ExternalInput").ap()
    bp_d = nc.dram_tensor("bp", [1, 4 * 128 * 64 * 8], BF16, kind="ExternalInput").ap()
    out_d = nc.dram_tensor("out", [4 * 128, 64 * 8], BF16, kind="ExternalOutput").ap()

    with tile.TileContext(nc) as tc:
        with (
            tc.tile_pool(name="cst", bufs=1) as cst,
            tc.tile_pool(name="work", bufs=1) as work,
            tc.tile_pool(name="trp", bufs=2) as trp,
            tc.tile_pool(name="wpp", bufs=N_TILES) as wpp,
            tc.tile_pool(name="outp", bufs=2) as outp,
            tc.tile_pool(name="bpp", bufs=2) as bpp,
            tc.tile_pool(name="ps_mm", bufs=1, space="PSUM") as ps_mm,
            tc.tile_pool(name="ps_tp", bufs=1, space="PSUM") as ps_tp,
            tc.tile_pool(name="ps_o", bufs=6, space="PSUM") as ps_o,
        ):
            # ---- phase 0: all input DMAs; qt/kt tiles split across both
            # HWDGE rings ahead of the wp stream ----
            wq_sb = cst.tile([128, 2, 256], F32)
            nc.sync.dma_start(wq_sb[:, :, :], wq_d.rearrange("(c p) d -> p c d", p=128))
            wk_sb = cst.tile([128, 2, 256], F32)
            nc.sync.dma_start(wk_sb[:, :, :], wk_d.rearrange("(c p) d -> p c d", p=128))
            wv_sb = cst.tile([128, 2, 256], F32)
            nc.sync.dma_start(wv_sb[:, :, :], wv_d.rearrange("(c p) d -> p c d", p=128))
            bq_sb = cst.tile([1, 256], F32)
            nc.sync.dma_start(bq_sb[:, :], bq_d)
            bk_sb = cst.tile([1, 256], F32)
            nc.sync.dma_start(bk_sb[:, :], bk_d)
            bv_sb = cst.tile([1, 256], F32)
            nc.sync.dma_start(bv_sb[:, :], bv_d)
            # per-batch transposed q/k tiles: sync carries q, scalar carries k
            trqs, trks = [], []
            for b in range(B):
                trq = trp.tile([128, 2, L], F32, tag="trq")
                nc.sync.dma_start(
                    trq[:, :, :],
                    qt_d[D * b:D * (b + 1), :].rearrange("(c p) l -> p c l", p=128))
                trqs.append(trq)
                trk = trp.tile([128, 2, L], F32, tag="trk")
                nc.scalar.dma_start(
                    trk[:, :, :],
                    kt_d[D * b:D * (b + 1), :].rearrange("(c p) l -> p c l", p=128))
                trks.append(trk)
            wpt = []
            for nt in range(N_TILES):
                ncol = slice(TILE_N * nt, TILE_N * (nt + 1))
                wp_t = wpp.tile([128, 2, TILE_N], BF16, tag="wp")
                eng = nc.sync if nt % 2 == 0 else nc.scalar
                eng.dma_start(
                    wp_t[:, :, :],
                    wp_d[:, ncol].rearrange("(c p) n -> p c n", p=128))
                wpt.append(wp_t)

            # ---- small constants ----
            ident8 = cst.tile([8, 8], F32)
            make_identity(nc, ident8[:, :])
            one1 = cst.tile([1, 1], F32)
            nc.vector.memset(one1[:, :], 1.0)
            sones = cst.tile([8, 1], F32)
            nc.vector.memset(sones[:, :], SCALE)

            # PE warm-up while the first qt/kt tiles are in flight
            ps_warm = ps_mm.tile([128, 512], F32, tag="mm")
            for _ in range(4):
                nc.tensor.matmul(ps_warm[:, 0:256], wq_sb[:, 0, 0:128], wq_sb[:, 0, :],
                                 start=True, stop=True)

            wqs = cst.tile([128, 2, 8], F32)
            nc.vector.reduce_sum(out=wqs[:, :, :],
                                 in_=wq_sb[:, :, :].rearrange("p c (h z) -> p c h z", z=DK),
                                 axis=mybir.AxisListType.X)
            wks = cst.tile([128, 2, 8], F32)
            nc.vector.reduce_sum(out=wks[:, :, :],
                                 in_=wk_sb[:, :, :].rearrange("p c (h z) -> p c h z", z=DK),
                                 axis=mybir.AxisListType.X)
            bqs_row = cst.tile([1, 8], F32)
            nc.vector.reduce_sum(out=bqs_row[:, :],
                                 in_=bq_sb[:, :].rearrange("o (h z) -> o h z", z=DK),
                                 axis=mybir.AxisListType.X)
            bks_row = cst.tile([1, 8], F32)
            nc.vector.reduce_sum(out=bks_row[:, :],
                                 in_=bk_sb[:, :].rearrange("o (h z) -> o h z", z=DK),
                                 axis=mybir.AxisListType.X)
            bqs_ps = ps_tp.tile([8, 1], F32, tag="tp")
            nc.tensor.matmul(bqs_ps[:, :], bqs_row[:, :], one1[:, :], start=True, stop=True)
            bqs_vert = cst.tile([8, 1], F32)
            nc.vector.tensor_copy(bqs_vert[:, :], bqs_ps[:, :])
            bks_ps = ps_tp.tile([8, 1], F32, tag="tp")
            nc.tensor.matmul(bks_ps[:, :], bks_row[:, :], one1[:, :], start=True, stop=True)
            bks_vert = cst.tile([8, 1], F32)
            nc.vector.tensor_copy(bks_vert[:, :], bks_ps[:, :])

            # ---- per-batch corr, top-6, softmax, 6-row gather, vbar ----
            vbar8s = work.tile([1, 8, 256], F32)
            for b in range(B):
                xsT = {}
                for (tr, w_sum, bias_v, nm) in (
                    (trqs[b], wqs, bqs_vert, "q"),
                    (trks[b], wks, bks_vert, "k"),
                ):
                    xs = work.tile([8, L], F32, tag=f"{nm}sT")
                    for half in range(2):
                        sl = slice(512 * half, 512 * (half + 1))
                        ps_x = ps_mm.tile([8, 512], F32, tag="mm")
                        nc.tensor.matmul(ps_x[:, :], w_sum[:, 0, :], tr[:, 0, sl], start=True, stop=False)
                        nc.tensor.matmul(ps_x[:, :], w_sum[:, 1, :], tr[:, 1, sl], start=False, stop=True)
                        nc.vector.tensor_scalar(
                            out=xs[:, sl], in0=ps_x[:, :],
                            scalar1=bias_v[:, 0:1], scalar2=None, op0=mybir.AluOpType.add)
                    xsT[nm] = xs
                prod = work.tile([8, L], F32, tag="prod")
                nc.vector.tensor_mul(prod[:, :], xsT["q"][:, :], xsT["k"][:, :])
                r_sb = work.tile([1, L], F32, tag="qsT")
                for half in range(2):
                    sl = slice(512 * half, 512 * (half + 1))
                    ps_r = ps_mm.tile([1, 512], F32, tag="mm")
                    nc.tensor.matmul(ps_r[:, :], sones[:, :], prod[:, sl], start=True, stop=True)
                    nc.vector.tensor_copy(r_sb[:, sl], ps_r[:, :])

                topv = work.tile([1, 8], F32, tag="topv")
                nc.vector.max(topv[:, :], r_sb[:, :])
                topi = work.tile([1, 8], U32, tag="topi")
                nc.vector.max_index(topi[:, :], topv[:, :], r_sb[:, :])
                negm0 = work.tile([1, 1], F32, tag="negm0")
                nc.vector.tensor_scalar_mul(negm0[:, :], topv[:, 0:1], -1.0)
                e_sb = work.tile([1, K_TOP], F32, tag="e_sb")
                nc.scalar.activation(e_sb[:, :], topv[:, 0:K_TOP],
                                     mybir.ActivationFunctionType.Exp,
                                     bias=negm0[:, 0:1], scale=1.0)
                z_sb = work.tile([1, 1], F32, tag="z_sb")
                nc.vector.reduce_sum(out=z_sb[:, :], in_=e_sb[:, :], axis=mybir.AxisListType.X)
                zinv = work.tile([1, 1], F32, tag="zinv")
                nc.vector.reciprocal(zinv[:, :], z_sb[:, :])
                w_sb = work.tile([1, K_TOP], F32, tag="w_sb")
                nc.vector.tensor_scalar_mul(w_sb[:, :], e_sb[:, :], zinv[:, 0:1])

                # indices (+1024*b since v spans all batches) -> gather 6 rows
                topi_f = work.tile([1, 8], F32, tag="topi_f")
                nc.vector.tensor_copy(topi_f[:, :], topi[:, :])
                nc.vector.tensor_scalar_add(topi_f[:, :], topi_f[:, :], float(L * b))
                idx_ps = ps_tp.tile([8, 1], F32, tag="tp")
                nc.tensor.matmul(idx_ps[:, :], topi_f[:, :], one1[:, :], start=True, stop=True)
                idx_colf = work.tile([8, 1], F32, tag="idx_colf", bufs=2)
                nc.vector.tensor_copy(idx_colf[:, :], idx_ps[:, :])
                idx_col = work.tile([8, 1], U32, tag="idx_col", bufs=2)
                nc.vector.tensor_copy(idx_col[:, :], idx_colf[:, :])
                w_ps = ps_tp.tile([K_TOP, 1], F32, tag="tp")
                nc.tensor.matmul(w_ps[:, :], w_sb[:, :], one1[:, :], start=True, stop=True)
                w_col = work.tile([K_TOP, 1], F32, tag="w_col", bufs=2)
                nc.vector.tensor_copy(w_col[:, :], w_ps[:, :])
                vrows = work.tile([K_TOP, 256], F32, tag="vrows", bufs=2)
                nc.gpsimd.indirect_dma_start(
                    out=vrows[:, :],
                    out_offset=None,
                    in_=v_d[:, :],
                    in_offset=bass.IndirectOffsetOnAxis(ap=idx_col[0:K_TOP, 0:1], axis=0),
                )
                vb_ps = ps_tp.tile([1, 256], F32, tag="tp")
                nc.tensor.matmul(vb_ps[:, :], w_col[:, :], vrows[:, :], start=True, stop=True)
                nc.vector.tensor_copy(vbar8s[:, b, :], vb_ps[:, :])

            # ---- partition-spread vbar, then Wv projection ----
            vbar8 = work.tile([8, 256], F32)
            nc.gpsimd.dma_start(vbar8[:, :], vbar8s[:, :, :].rearrange("o b d -> (o b) d"))
            vbarT = work.tile([128, 2, 8], F32)
            for m in range(2):
                pt = ps_tp.tile([128, 8], F32, tag="tp")
                nc.tensor.transpose(pt[:, :], vbar8[0:8, 128 * m:128 * (m + 1)], ident8[:, :])
                nc.vector.tensor_copy(vbarT[:, m, :], pt[:, :])
            # aggf[8, 256] = vbar @ Wv + bv
            ps_a = ps_mm.tile([8, 512], F32, tag="mm")
            nc.tensor.matmul(ps_a[:, 0:256], vbarT[:, 0, :], wv_sb[:, 0, :], start=True, stop=False)
            nc.tensor.matmul(ps_a[:, 0:256], vbarT[:, 1, :], wv_sb[:, 1, :], start=False, stop=False)
            ones8m = cst.tile([1, 8], F32)
            nc.vector.memset(ones8m[:, :], 1.0)
            nc.tensor.matmul(ps_a[:, 0:256], ones8m[:, :], bv_sb[:, :], start=False, stop=True)
            aggf = cst.tile([8, 256], F32)
            nc.vector.tensor_copy(aggf[:, :], ps_a[:, 0:256])
            aggt_bf = cst.tile([128, 16], BF16)
            for m in range(2):
                pt2 = ps_tp.tile([128, 8], F32, tag="tp")
                nc.tensor.transpose(pt2[:, :], aggf[0:8, 128 * m:128 * (m + 1)], ident8[:, :])
                nc.vector.tensor_copy(aggt_bf[:, 8 * m:8 * (m + 1)], pt2[:, :])

            # ---- big output projection, transposed (see dp build) ----
            bp_sb = cst.tile([128, 4, 64, 8], BF16)
            nc.sync.dma_start(bp_sb[:, :, :, :], bp_d.rearrange("o (p t c b) -> (o p) t c b", t=4, p=128, c=64))
            for t in range(4):
                ps = ps_o.tile([128, 64, 8], F32, tag="po")
                for c in range(64):
                    n0 = 8192 * t + 128 * c
                    wt = wpt[n0 // TILE_N]
                    co = n0 % TILE_N
                    nc.tensor.matmul(ps[:, c, :], wt[:, 0, co:co + 128], aggt_bf[:, 0:8],
                                     start=True, stop=False)
                    nc.tensor.matmul(ps[:, c, :], wt[:, 1, co:co + 128], aggt_bf[:, 8:16],
                                     start=False, stop=True)
                o_sbT = outp.tile([128, 64, 8], BF16)
                nc.vector.tensor_add(o_sbT[:, :, :], ps[:, :, :], bp_sb[:, t, :, :])
                nc.gpsimd.dma_start(out_d[128 * t:128 * (t + 1), :], o_sbT[:, :, :].rearrange("p c b -> p (c b)"))

    nc.finalize()
    return nc


def _get_nc():
    if "nc" not in _CACHE:
        _CACHE["nc"] = _build_nc_repl() if MODE == "repl" else _build_nc()
    return _CACHE["nc"]


def kernel(queries, keys, values, Wq, bq, Wk, bk, Wv, bv, Wp, bp):
    queries = np.asarray(queries, np.float32)
    keys = np.asarray(keys, np.float32)
    values = np.asarray(values, np.float32)
    Wq = np.ascontiguousarray(np.asarray(Wq, np.float32))
    Wk = np.ascontiguousarray(np.asarray(Wk, np.float32))
    Wv = np.ascontiguousarray(np.asarray(Wv, np.float32))
    bq = np.asarray(bq, np.float32).reshape(1, D)
    bk = np.asarray(bk, np.float32).reshape(1, D)
    bv = np.asarray(bv, np.float32).reshape(1, D)
    Wp = np.asarray(Wp, np.float32)
    bp = np.asarray(bp, np.float32)

    nc = _get_nc()
    qT = np.ascontiguousarray(queries.transpose(0, 2, 1))   # [B, D, L]
    kT = np.ascontiguousarray(keys.transpose(0, 2, 1))
    Wp_bf = np.asarray(Wp, dtype=ml_dtypes.bfloat16)
    in_maps = []
    for i in range(N_CORES):
        cols = slice(NSH * i, NSH * (i + 1))
        # bias pre-scrambled to the transposed-output layout [p, t, c, b]
        # (value at flat (t, p, c, b) = bp[8192 t + 128 c + p], replicated over b)
        bp_shard = np.asarray(bp[cols], np.float32).reshape(4, 64, 128)
        bp_scr = np.broadcast_to(
            bp_shard.transpose(2, 0, 1)[:, :, :, None], (128, 4, 64, 8))
        m = {
            "wq": Wq, "wk": Wk, "wv": Wv,
            "bq": bq, "bk": bk, "bv": bv,
            "wp": np.ascontiguousarray(Wp_bf[:, cols]),
            "bp": np.ascontiguousarray(
                np.asarray(bp_scr, dtype=ml_dtypes.bfloat16)).reshape(1, 4 * 128 * 64 * 8),
        }
        if MODE == "repl":
            m.update({"qt": qT.reshape(B * D, L), "kt": kT.reshape(B * D, L),
                      "v": values.reshape(B * L, D)})
        else:
            b0 = 2 * (i // 2)
            m.update({"qt0": qT[b0], "kt0": kT[b0], "v0": values[b0],
                      "qt1": qT[b0 + 1], "kt1": kT[b0 + 1], "v1": values[b0 + 1]})
        in_maps.append(m)
    res = run_bass_kernel_spmd(nc, in_maps, core_ids=list(range(N_CORES)), trace=TRACE)
    global LAST_RESULT
    LAST_RESULT = res
    shards = []
    for i in range(N_CORES):
        buf = np.asarray(res.results[i]["out"], np.float32)
        if buf.shape == (4 * 128, 64 * 8):
            # transposed layout: buf[(t p), (c b)] -> shard[b, 8192 t + 128 c + p]
            shards.append(buf.reshape(4, 128, 64, 8).transpose(3, 0, 2, 1).reshape(B, NSH))
        else:
            shards.append(buf)
    out = np.concatenate(shards, axis=1)
    return out.reshape(B, L, D)
